# revision 26
# baseline (speedup 1.0000x reference)
"""Trainium2 Bass kernel for nn_FPSWE_40303973105696.

Computation (see problem reference): project X onto P directions, sort along
N, linearly interpolate N->M quantiles, subtract from ref, contract with
weight.

Algebraic folding done on host:
    out[b, p] = rw[p] - sum_n Xs[b, n, p] * W2[p, n]
where
    rw[p]    = sum_m ref[m] * weight[p, m]
    W2[p, n] = interpolation matrix folded into weight (fixed scatter)
    Xs       = sort_n(X @ theta_w.T)

Device kernel per core (data-parallel over B, core c handles batch c):
    1. proj[p, n] = theta_w[p, :] @ X[b].T        (PE, fp16 in / fp32 PSUM)
    2. sort proj rows along n (free axis)         (bitonic, 66 stages, fp16)
    3. acc[p] = sum_n Xs[p, n] * W2[p, n]         (DVE mul + ACT accum)
    4. out[p] = rw[p] - acc[p]

The sort runs entirely on DVE in fp16 (GPSIMD's tensor_tensor ucode has no
min/max, so Pool cannot compare-exchange). fp16 tensor_tensor hits the 2x_1P
perf mode (two packed 16-bit lanes per port read) only when every operand's
innermost run is >= 2 packed elements, so elements are stored in a
bit-rotated physical layout: logical sort-index bit j lives at physical bit
j+1 and logical bit 10 at physical bit 0. Every compare-exchange at logical
stride 2^j then pairs elements at physical stride 2^(j+1) (innermost run
2^(j+1) >= 2), and mirror stages keep physical bit 0 as a forward inner
dim of 2, so all 66 stages except the final-level mirror run at 2x. fp16
quantization of the projections (rel ~5e-4) is far inside the 2e-2 gate.
The physical->logical permutation is folded into W2 on the host.
"""

import numpy as np

from concourse import bass, bacc, mybir
from concourse.tile import TileContext
from concourse.bass_utils import run_bass_kernel_spmd

B, N, D, P, M = 8, 2048, 128, 1024, 1024
NT = P // 128          # 8 projection row-chunks of 128 partitions each
MM_CHUNK = 512         # matmul free-dim chunk (one PSUM bank)
N_CORES = 8

FP = mybir.dt.float32
FP16 = mybir.dt.float16

# row-groups: (num_row_chunks, engine_name, sort_dtype). Sum must be NT.
GROUPS = [(NT, "vector", FP16)]

# debug knob: limit number of sort stages emitted (None = all)
STAGE_LIMIT = None
# benchmark knob: emit the whole kernel body this many times (timing only)
REPEAT = 1


def _sort_stages(n):
    """Uniform-direction bitonic network in the bit-rotated physical layout.

    ("mirror", L) = first stage of the merge at logical size 2^L;
    ("std", 2*st) = stride-st stage, emitted at physical stride 2*st.
    66 stages for n=2048.
    """
    stages = []
    size = 2
    while size <= n:
        stages.append(("mirror", size.bit_length() - 1))
        st = size // 4
        while st >= 1:
            stages.append(("std", 2 * st))
            st //= 2
        size *= 2
    return stages


def _emit_sort_stage(eng, kind, val, cur, oth):
    """One compare-exchange stage: read cur AP, write oth AP (same shape).

    Physical layout: logical bit j at phys bit j+1 (j=0..9), logical bit 10
    at phys bit 0. Mirror at logical size 2^L complements phys bits 1..L and
    keeps phys bit 0 as a forward inner dim, so reads/writes stay packed.
    """
    if kind == "mirror":
        L = val
        if L <= 10:
            r = 1 << (L - 1)
            v = cur.rearrange("p (n t r b) -> p n t r b", t=2, r=r, b=2)
            o = oth.rearrange("p (n t r b) -> p n t r b", t=2, r=r, b=2)
            eng.tensor_tensor(o[:, :, 0], v[:, :, 0], v[:, :, 1, ::-1],
                              op=mybir.AluOpType.min)
            eng.tensor_tensor(o[:, :, 1], v[:, :, 0, ::-1], v[:, :, 1],
                              op=mybir.AluOpType.max)
        else:
            v = cur.rearrange("p (c r b) -> p c r b", r=1024, b=2)
            o = oth.rearrange("p (c r b) -> p c r b", r=1024, b=2)
            eng.tensor_tensor(o[:, :, :, 0], v[:, :, :, 0], v[:, :, ::-1, 1],
                              op=mybir.AluOpType.min)
            eng.tensor_tensor(o[:, :, :, 1], v[:, :, ::-1, 0], v[:, :, :, 1],
                              op=mybir.AluOpType.max)
    else:
        stp = val
        v = cur.rearrange("p (n t s) -> p n t s", t=2, s=stp)
        o = oth.rearrange("p (n t s) -> p n t s", t=2, s=stp)
        eng.tensor_tensor(o[:, :, 0, :], v[:, :, 0, :], v[:, :, 1, :],
                          op=mybir.AluOpType.min)
        eng.tensor_tensor(o[:, :, 1, :], v[:, :, 0, :], v[:, :, 1, :],
                          op=mybir.AluOpType.max)


def _build_kernel():
    assert sum(g for g, _, _ in GROUPS) == NT
    nc = bacc.Bacc()

    xt = nc.declare_dram_parameter("xt", [D, N], FP16, isOutput=False)     # X[b].T
    tht = nc.declare_dram_parameter("tht", [D, P], FP16, isOutput=False)   # theta_w.T
    rw = nc.declare_dram_parameter("rw", [128, NT], FP, isOutput=False)    # rw[p] as [128, 8]
    out = nc.declare_dram_parameter("out", [128, NT], FP, isOutput=True)
    # folded weight, one DRAM param per group (dtype matches the group's sort
    # dtype so the group's engine can run the final elementwise dot itself)
    w2_params = []
    t0 = 0
    for gi, (gsz, _, dt) in enumerate(GROUPS):
        w2_params.append(
            nc.declare_dram_parameter(f"w2g{gi}", [gsz * 128, N], dt,
                                      isOutput=False))
        t0 += gsz

    stages = _sort_stages(N)
    if STAGE_LIMIT is not None:
        stages = stages[:STAGE_LIMIT]

    with TileContext(nc) as tc:
        with (
            tc.tile_pool(name="const", bufs=1) as const_pool,
            tc.tile_pool(name="xt", bufs=1) as xt_pool,
            tc.tile_pool(name="sa", bufs=1) as a_pool,
            tc.tile_pool(name="sb", bufs=1) as b_pool,
            tc.tile_pool(name="w2", bufs=NT) as w2_pool,
            tc.tile_pool(name="ps", bufs=2, space="PSUM") as psum_pool,
        ):
            tht_sb = const_pool.tile([D, P], FP16, tag="tht")
            tht_raw = const_pool.tile([D, P], FP16, tag="thtr")
            rw_sb = const_pool.tile([128, NT], FP, tag="rw")
            acc_sb = const_pool.tile([128, NT], FP, tag="acc")
            out_sb = const_pool.tile([128, NT], FP, tag="outsb")
            out_tmp = const_pool.tile([128, NT], FP, tag="outtmp")
            xt_sb = xt_pool.tile([D, N], FP16, tag="xt")

            # Bounce tht through ACT so Matmult instructions never carry two
            # DMA-queue semaphore waits (walrus codegen limit); xt can then
            # DMA straight into SBUF (one DMA wait + one ACT wait is fine).
            nc.sync.dma_start(out=tht_raw[:], in_=tht[:])
            nc.sync.dma_start(out=rw_sb[:], in_=rw[:])
            nc.sync.dma_start(out=xt_sb[:], in_=xt[:])
            nc.scalar.copy(out=tht_sb[:], in_=tht_raw[:])

            a_tiles, b_tiles = {}, {}
            for gi, (gsz, _, dt) in enumerate(GROUPS):
                a_tiles[gi] = a_pool.tile([128, gsz * N], dt,
                                          tag=f"a{gi}", name=f"a{gi}")
                b_tiles[gi] = b_pool.tile([128, gsz * N], dt,
                                          tag=f"b{gi}", name=f"b{gi}")

            def emit_body(rep_i):
                t0 = 0
                for gi, (gsz, eng_name, dt) in enumerate(GROUPS):
                    a_t, b_t = a_tiles[gi], b_tiles[gi]
                    eng = getattr(nc, eng_name)
                    w2_p = w2_params[gi]

                    # prefetch all w2 tiles (independent of compute) so the
                    # final-dot muls never stall on DMA
                    w2_tiles = []
                    for r in range(gsz):
                        w2_sb = w2_pool.tile([128, N], dt,
                                             tag=f"w2g{gi}", name=f"w2g{gi}")
                        nc.sync.dma_start(
                            out=w2_sb[:], in_=w2_p[r * 128:(r + 1) * 128, :])
                        w2_tiles.append(w2_sb)

                    # ---- projection matmuls for this group's row chunks ----
                    for r in range(gsz):
                        t = t0 + r
                        ps = psum_pool.tile([128, N], FP, tag="ps", name="ps")
                        for ch in range(N // MM_CHUNK):
                            nc.tensor.matmul(
                                ps[:, ch * MM_CHUNK:(ch + 1) * MM_CHUNK],
                                lhsT=tht_sb[:, t * 128:(t + 1) * 128],
                                rhs=xt_sb[:, ch * MM_CHUNK:(ch + 1) * MM_CHUNK],
                                start=True, stop=True,
                            )
                        # PSUM fp32 -> SBUF (cast to the group's sort dtype)
                        nc.scalar.copy(out=a_t[:, r * N:(r + 1) * N], in_=ps[:])

                    # ---- bitonic sort along free axis (ends back in a_t) ----
                    # The first TWO stages are emitted per row-chunk so DVE
                    # has ~2.1us of work per arriving PSUM cast (the ACT cast
                    # stream runs at ~1.9us/chunk; one stage per chunk would
                    # starve DVE). The last stage is also per-chunk so each
                    # chunk's weighted reduction starts right away.
                    cur, oth = a_t[:], b_t[:]
                    for r in range(gsz):
                        sl = slice(r * N, (r + 1) * N)
                        _emit_sort_stage(eng, stages[0][0], stages[0][1],
                                         cur[:, sl], oth[:, sl])
                        _emit_sort_stage(eng, stages[1][0], stages[1][1],
                                         oth[:, sl], cur[:, sl])
                    for si in range(2, len(stages)):
                        stg = stages[si]
                        last = si == len(stages) - 1
                        if last:
                            for r in range(gsz):
                                sl = slice(r * N, (r + 1) * N)
                                _emit_sort_stage(eng, stg[0], stg[1],
                                                 cur[:, sl], oth[:, sl])
                                t = t0 + r
                                # weighted dot on the sort engine (a Pool
                                # mul is 4x slower, and a fused
                                # scalar_tensor_tensor+accum runs at 1x —
                                # both lose to fp16 TT at 2x); ACT reduces
                                # via accum_out, 'cur' is a dead dump
                                eng.tensor_mul(cur[:, sl],
                                               oth[:, sl], w2_tiles[r][:])
                                nc.scalar.activation(
                                    oth[:, sl], cur[:, sl],
                                    mybir.ActivationFunctionType.Copy,
                                    accum_out=acc_sb[:, t:t + 1])
                        else:
                            _emit_sort_stage(eng, stg[0], stg[1], cur, oth)
                        cur, oth = oth, cur
                    t0 += gsz

                # accumulate across repeat bodies so none is dead code;
                # the final output is REPEAT * (rw - acc), divided on host
                if rep_i == 0:
                    nc.vector.tensor_sub(out_sb[:], rw_sb[:], acc_sb[:])
                else:
                    nc.vector.tensor_sub(out_tmp[:], rw_sb[:], acc_sb[:])
                    nc.vector.tensor_add(out_sb[:], out_sb[:], out_tmp[:])

            for _rep in range(REPEAT):
                emit_body(_rep)
            nc.sync.dma_start(out=out[:], in_=out_sb[:])

    return nc


_NC_CACHE = None


def _get_nc():
    global _NC_CACHE
    if _NC_CACHE is None:
        nc = _build_kernel()
        nc.finalize()   # Bacc: runs wait-splitting + register allocation
        _NC_CACHE = nc
    return _NC_CACHE


def _host_precompute(X, theta_w, ref, weight):
    X = np.ascontiguousarray(np.asarray(X, dtype=np.float32))
    xt = np.ascontiguousarray(X.transpose(0, 2, 1).astype(np.float16))  # [B, D, N] fp16
    tht, w2_groups, rw_sb = _const_precompute(theta_w, ref, weight)
    return xt, tht, w2_groups, rw_sb


def _in_maps(X, theta_w, ref, weight):
    xt, tht, w2_groups, rw_sb = _host_precompute(X, theta_w, ref, weight)
    base = {"tht": tht, "rw": rw_sb}
    for gi, w2g in enumerate(w2_groups):
        base[f"w2g{gi}"] = w2g
    return [{**base, "xt": xt[c]} for c in range(N_CORES)]


def _const_precompute(theta_w, ref, weight):
    """Everything derived from (theta_w, ref, weight) — X-independent."""
    theta_w = np.asarray(theta_w, dtype=np.float32)
    ref = np.asarray(ref, dtype=np.float32)
    weight = np.asarray(weight, dtype=np.float32)

    tht = np.ascontiguousarray(theta_w.T.astype(np.float16))  # [D, P] fp16

    x1d = np.linspace(0.0, 1.0, N + 2, dtype=np.float32)[1:-1]
    xnew = np.linspace(0.0, 1.0, M + 2, dtype=np.float32)[1:-1]
    ind = np.clip(np.searchsorted(x1d, xnew) - 1, 0, N - 2)
    eps = np.float32(np.finfo(np.float32).eps)
    dx = x1d[1:] - x1d[:-1]
    t32 = ((xnew - x1d[ind]) / (eps + dx[ind])).astype(np.float32)

    # ind and ind+1 are all distinct and together cover 0..N-1 exactly once,
    # so the interpolation fold is a permuted assignment (no accumulation).
    w2nt = np.zeros((N, P), dtype=np.float32)                # [N, P]
    w2nt[ind] = (1.0 - t32)[:, None] * weight.T
    w2nt[ind + 1] = t32[:, None] * weight.T
    w2 = np.ascontiguousarray(w2nt.T)                        # [P, N]

    # fold the physical bit-rotated sort layout into W2: phys slot f holds
    # the e(f)-th order statistic, e(f) = (f >> 1) | ((f & 1) << 10)
    f = np.arange(N)
    e_of_f = (f >> 1) | ((f & 1) << (N.bit_length() - 2))
    w2 = w2[:, e_of_f]

    w2_groups = []
    t0 = 0
    for gsz, _, dt in GROUPS:
        sl = w2[t0 * 128:(t0 + gsz) * 128, :]
        w2_groups.append(np.ascontiguousarray(sl.astype(mybir.dt.np(dt))))
        t0 += gsz

    rw = (weight.astype(np.float64) @ ref.astype(np.float64)).astype(np.float32)
    rw_sb = np.ascontiguousarray(rw.reshape(NT, 128).T)      # [128, NT]
    return tht, w2_groups, rw_sb


def _const_fingerprint(theta_w, ref, weight):
    import hashlib
    h = hashlib.blake2b(digest_size=16)
    for a in (theta_w, ref, weight):
        a = np.ascontiguousarray(a)
        h.update(str(a.shape).encode())
        h.update(a.tobytes())
    return h.hexdigest()


_RUNNER = None          # cached (fingerprint, consts, jit runner state)


def _axon_active():
    try:
        from concourse._compat import axon_active
        return axon_active()
    except Exception:
        try:
            import jax
            return any("axon" in str(d.platform).lower() for d in jax.devices())
        except Exception:
            return False


def _make_runner(consts):
    """Cached jit executable with device-resident constant inputs.

    Per call only the X-derived input ("xt") is shipped; constants
    (tht / w2 groups / rw) and output seed buffers stay on device.
    """
    import jax
    from jax.sharding import Mesh, PartitionSpec
    from jax.experimental.shard_map import shard_map
    from concourse import mybir as _mybir
    from concourse.bass2jax import (
        _bass_exec_p, install_neuronx_cc_hook, partition_id_tensor,
    )

    install_neuronx_cc_hook()
    nc = _get_nc()
    tht, w2_groups, rw_sb = consts
    const_map = {"tht": tht, "rw": rw_sb}
    for gi, w2g in enumerate(w2_groups):
        const_map[f"w2g{gi}"] = w2g

    partition_name = (nc.partition_id_tensor.name
                      if nc.partition_id_tensor else None)
    in_names, out_names, out_avals, zero_outs = [], [], [], []
    for alloc in nc.m.functions[0].allocations:
        if not isinstance(alloc, _mybir.MemoryLocationSet):
            continue
        name = alloc.memorylocations[0].name
        if alloc.kind == "ExternalInput":
            if name != partition_name:
                in_names.append(name)
        elif alloc.kind == "ExternalOutput":
            out_names.append(name)
            shape = tuple(alloc.tensor_shape)
            dtype = _mybir.dt.np(alloc.dtype)
            out_avals.append(jax.core.ShapedArray(shape, dtype))
            zero_outs.append(np.zeros(shape, dtype))
    n_params = len(in_names)
    all_names = in_names + out_names
    if partition_name is not None:
        all_names = all_names + [partition_name]

    def _body(*args):
        operands = list(args)
        if partition_name is not None:
            operands.append(partition_id_tensor())
        outs = _bass_exec_p.bind(
            *operands,
            out_avals=tuple(out_avals),
            in_names=tuple(all_names),
            out_names=tuple(out_names),
            lowering_input_output_aliases=(),
            sim_require_finite=True,
            sim_require_nnan=True,
            nc=nc,
        )
        return tuple(outs)

    devices = jax.devices()[:N_CORES]
    mesh = Mesh(np.asarray(devices), ("core",))
    nin = n_params + len(zero_outs)
    fn = jax.jit(
        shard_map(_body, mesh=mesh,
                  in_specs=(PartitionSpec("core"),) * nin,
                  out_specs=(PartitionSpec("core"),) * len(out_names),
                  check_rep=False),
        keep_unused=True,
    )

    # device-resident inputs: constants replicated per core, concat on axis 0
    # and placed WITH the mesh sharding (an unsharded put would be re-sharded
    # device-to-device at every dispatch)
    sharding = jax.sharding.NamedSharding(mesh, PartitionSpec("core"))
    dev_cache = {}
    for nm in in_names:
        if nm in const_map:
            a = const_map[nm]
            dev_cache[nm] = jax.device_put(
                np.concatenate([a] * N_CORES, axis=0), sharding)
    dev_zero = [jax.device_put(
        np.concatenate([z] * N_CORES, axis=0), sharding) for z in zero_outs]

    def run(xt_all):
        """xt_all: [B, D, N] fp32 (per-core xt slices)."""
        args = []
        for nm in in_names:
            if nm == "xt":
                args.append(jax.device_put(
                    np.ascontiguousarray(xt_all.reshape(B * D, N)), sharding))
            else:
                args.append(dev_cache[nm])
        outs = fn(*args, *dev_zero)
        jax.block_until_ready(outs)
        o = np.asarray(outs[out_names.index("out")])
        out_full = np.empty((B, P), dtype=np.float32)
        for c in range(N_CORES):
            out_full[c] = np.ascontiguousarray(
                o[c * 128:(c + 1) * 128].T).reshape(P)
        return out_full

    return run


def kernel(X, theta_w, ref, weight):
    import time as _time
    global _RUNNER

    X = np.asarray(X, dtype=np.float32)
    xt_all = np.ascontiguousarray(
        X.transpose(0, 2, 1).astype(np.float16))             # [B, D, N] fp16

    if _axon_active():
        fp = _const_fingerprint(theta_w, ref, weight)
        last_err = None
        for attempt in range(3):
            try:
                if _RUNNER is None or _RUNNER[0] != fp:
                    consts = _const_precompute(theta_w, ref, weight)
                    _RUNNER = (fp, _make_runner(consts))
                return _RUNNER[1](xt_all)
            except Exception as e:   # transient transport errors
                last_err = e
                _RUNNER = None
                _time.sleep(5)
        raise last_err

    # native (non-axon) fallback: classic spmd path
    nc = _get_nc()
    in_maps = _in_maps(X, theta_w, ref, weight)
    last_err = None
    for attempt in range(3):
        try:
            res = run_bass_kernel_spmd(nc, in_maps, list(range(N_CORES)))
            break
        except Exception as e:  # transient transport errors (mesh desync)
            last_err = e
            _time.sleep(5)
    else:
        raise last_err
    outs = res.results if hasattr(res, "results") else res
    out_full = np.empty((B, P), dtype=np.float32)
    for c in range(N_CORES):
        out_full[c] = np.ascontiguousarray(outs[c]["out"].T).reshape(P)
    return out_full


# ---------------------------------------------------------------------------
# Benchmark path: cached jit + device-resident inputs, excludes host transfer.
# ---------------------------------------------------------------------------

def make_bench(X, theta_w, ref, weight):
    import jax
    from jax.sharding import Mesh, PartitionSpec
    from jax.experimental.shard_map import shard_map
    from concourse import bass2jax, mybir as _mybir
    from concourse.bass2jax import (
        _bass_exec_p, install_neuronx_cc_hook, partition_id_tensor,
    )

    install_neuronx_cc_hook()
    nc = _get_nc()
    in_maps = _in_maps(X, theta_w, ref, weight)

    partition_name = (nc.partition_id_tensor.name
                      if nc.partition_id_tensor else None)
    in_names, out_names, out_avals, zero_outs = [], [], [], []
    for alloc in nc.m.functions[0].allocations:
        if not isinstance(alloc, _mybir.MemoryLocationSet):
            continue
        name = alloc.memorylocations[0].name
        if alloc.kind == "ExternalInput":
            if name == partition_name:
                continue
            in_names.append(name)
        elif alloc.kind == "ExternalOutput":
            out_names.append(name)
            shape = tuple(alloc.tensor_shape)
            dtype = _mybir.dt.np(alloc.dtype)
            out_avals.append(jax.core.ShapedArray(shape, dtype))
            zero_outs.append(np.zeros(shape, dtype))
    n_params = len(in_names)
    all_names = in_names + out_names
    if partition_name is not None:
        all_names = all_names + [partition_name]

    def _body(*args):
        operands = list(args)
        if partition_name is not None:
            operands.append(partition_id_tensor())
        outs = _bass_exec_p.bind(
            *operands,
            out_avals=tuple(out_avals),
            in_names=tuple(all_names),
            out_names=tuple(out_names),
            lowering_input_output_aliases=(),
            sim_require_finite=True,
            sim_require_nnan=True,
            nc=nc,
        )
        return tuple(outs)

    devices = jax.devices()[:N_CORES]
    mesh = Mesh(np.asarray(devices), ("core",))
    nin = n_params + len(zero_outs)
    fn = jax.jit(
        shard_map(_body, mesh=mesh,
                  in_specs=(PartitionSpec("core"),) * nin,
                  out_specs=(PartitionSpec("core"),) * len(out_names),
                  check_rep=False),
        keep_unused=True,
    )
    per_core = [[np.asarray(m[nm]) for nm in in_names] for m in in_maps]
    concat_in = [
        np.concatenate([per_core[c][i] for c in range(N_CORES)], axis=0)
        for i in range(n_params)
    ] + [
        np.concatenate([z for _ in range(N_CORES)], axis=0) for z in zero_outs
    ]
    dev_in = [jax.device_put(a) for a in concat_in]

    def run():
        outs = fn(*dev_in)
        jax.block_until_ready(outs)
        return outs

    def collect(outs):
        arrs = [np.asarray(o) for o in outs]
        out_full = np.empty((B, P), dtype=np.float32)
        o = arrs[0] / REPEAT  # bodies accumulate REPEAT copies of the result
        for c in range(N_CORES):
            out_full[c] = np.ascontiguousarray(o[c * 128:(c + 1) * 128].T).reshape(P)
        return out_full

    return run, collect
